# revision 3
# baseline (speedup 1.0000x reference)
"""BesselConv2d Trainium2 kernel (bf16 direct conv, 41 taps-pair matmuls).

Math (matches reference):
  wr = T_real @ w_r - T_imag @ w_i          (M, K^2, Cin*Cout)
  wi = T_real @ w_i + T_imag @ w_r
  Wf = einops to (2*M*Cout, Cin, 9, 9) filter bank
  y  = conv2d(x, Wf, SAME)                  (N, 2048, 64, 64)
  out = square(y).reshape(N,2,M,Cout,H,W).sum((1,2)) + b

Device strategy (8 cores, data-parallel over batch: 4 images/core):
  Direct conv as PSUM-accumulated bf16 matmuls, contraction 128 =
  64 ci x 2 taps. The 2 taps are baked into the partition halves via
  compacted 64-wide plane copies: A half (partitions 0-63) holds
  copies for kx shifts {0,2,4,6,8}, B half holds {1,3,5,7, 8-rowshift}.
  Full MMs (ky, g): taps (ky, 2g), (ky, 2g+1), g=0..3; tail MMs pair
  (ky,8) with (ky+1,8) via the row-shifted copy. 9*4 + 5 = 41 MMs per
  (octile, pixel tile) — the 81-tap floor. Every rhs window is a plain
  contiguous 512-element slice (8 rows x 64 cols). M = 128 output
  channels (2 cm x 64 cout), N = 512 pixels. bf16 enables standalone
  LDWEIGHTS + FWL (fp32r self-loads weights serially: 320 ns/MM vs
  213 ns stream). Square on ScalarE, accumulate over the 16 octiles on
  VectorE, fold the 2 cm partition-halves + bias, DMA out.

Weight prep (filter bank + matmul-tile layout) is host-side numpy.
"""

import numpy as np
import ml_dtypes

BF16 = ml_dtypes.bfloat16

N_CORES = 8
N, CIN, H, W = 32, 64, 64, 64
COUT = 64
M_FREQ = 16
K = 9
PW = 72                 # padded plane height/width (64 + 2*4)
XSLOT = PW * 64         # one compacted plane copy: 72 rows x 64 cols = 4608
XBY = 5 * XSLOT        # 5 slots per partition = 23040 elements
NMM = 41                # matmuls per octile: 36 full + 5 tails
NIMG = N // N_CORES     # images per core
NOCT = 16               # octiles of 128 output channels (2048 total)
WO = NMM * 128          # weight elems per octile per partition row

# MM descriptors: (slot element offset, ky row offset)
_DESCS = [(g * XSLOT, ky) for ky in range(K) for g in range(4)]
_DESCS += [(4 * XSLOT, 2 * u) for u in range(5)]


def _host_prep(x, T_real, T_imag, w_r, w_i, b):
    # filter bank, exactly as the reference builds it
    wr = np.matmul(T_real, w_r) - np.matmul(T_imag, w_i)
    wi = np.matmul(T_real, w_i) + np.matmul(T_imag, w_r)
    Wf = np.stack([wr, wi], axis=0).reshape(2, M_FREQ, K, K, CIN, COUT)
    Wf = Wf.transpose(0, 1, 5, 4, 2, 3).reshape(2 * M_FREQ * COUT, CIN, K, K)

    # weights: Wq[(s,ci), o, idx, (q,cout)]
    Wv6 = Wf.reshape(NOCT, 2, COUT, CIN, K, K)   # (o, q, cout, ci, ky, kx)
    Wq = np.zeros((2, CIN, NOCT, NMM, 2, COUT), np.float32)
    for ky in range(K):
        for g in range(4):
            idx = ky * 4 + g
            for s in range(2):
                kx = 2 * g + s
                Wq[s, :, :, idx, :, :] = Wv6[:, :, :, :, ky, kx].transpose(3, 0, 1, 2)
    for u in range(5):
        for s in range(2):
            ky = 2 * u + s
            if ky <= 8:
                Wq[s, :, :, 36 + u, :, :] = Wv6[:, :, :, :, ky, 8].transpose(3, 0, 1, 2)
    w_flat = Wq.reshape(128, NOCT * WO).astype(BF16)

    # x: compacted 64-wide plane copies per kx shift
    xpad = np.zeros((N, CIN, PW, PW), np.float32)
    xpad[:, :, 4:68, 4:68] = x
    x16 = xpad.astype(BF16)
    xflat = np.zeros((N, 128, XBY), BF16)
    for j in range(5):
        xflat[:, 0:64, j * XSLOT:(j + 1) * XSLOT] = \
            x16[:, :, :, 2 * j:2 * j + 64].reshape(N, CIN, XSLOT)
    for j in range(4):
        xflat[:, 64:128, j * XSLOT:(j + 1) * XSLOT] = \
            x16[:, :, :, 2 * j + 1:2 * j + 65].reshape(N, CIN, XSLOT)
    # B half slot 4: kx=8 copy shifted up one row (for the ky-pair tails)
    c8r = np.zeros((N, CIN, PW, 64), BF16)
    c8r[:, :, :PW - 1] = x16[:, :, 1:, 8:72]
    xflat[:, 64:128, 4 * XSLOT:5 * XSLOT] = c8r.reshape(N, CIN, XSLOT)

    return xflat, w_flat, np.asarray(b, np.float32).reshape(COUT, 1)


_PROGRAM_CACHE = {}


def _build_program(repeat=1):
    key = repeat
    if key in _PROGRAM_CACHE:
        return _PROGRAM_CACHE[key]

    import concourse.tile as tile
    from concourse import bacc, mybir

    nc = bacc.Bacc("TRN2", target_bir_lowering=False, debug=False)
    F16 = mybir.dt.bfloat16
    F32 = mybir.dt.float32
    x_d = nc.dram_tensor("x", [NIMG, 128, XBY], F16, kind="ExternalInput").ap()
    w_d = nc.dram_tensor("w", [128, NOCT * WO], F16, kind="ExternalInput").ap()
    b_d = nc.dram_tensor("b", [COUT, 1], F32, kind="ExternalInput").ap()
    out_d = nc.dram_tensor("out", [NIMG, COUT, H * W], F32,
                           kind="ExternalOutput").ap()

    from contextlib import nullcontext

    with tile.TileContext(nc) as tc:
        with (
            tc.tile_pool(name="xpool", bufs=2) as xpool,
            tc.tile_pool(name="wpool", bufs=2) as wpool,
            tc.tile_pool(name="accp", bufs=8) as accp,
            tc.tile_pool(name="ps", bufs=8, space="PSUM") as ps,
            tc.tile_pool(name="sq", bufs=3) as sqp,
            tc.tile_pool(name="fold", bufs=3) as foldp,
            tc.tile_pool(name="singles", bufs=1) as singles,
        ):
            bt = singles.tile([COUT, 1], F32)
            nc.sync.dma_start(out=bt[:], in_=b_d)

            rep_ctx = (tc.For_i(0, repeat, 1, hint_engines=(mybir.EngineType.PE,))
                       if repeat > 1 else nullcontext())
            with rep_ctx:
                for n in range(NIMG):
                    xt = xpool.tile([128, XBY], F16)
                    nc.sync.dma_start(out=xt[:], in_=x_d[n])

                    accs = [accp.tile([128, 512], F32, name=f"acc{_t}", tag="acc")
                            for _t in range(8)]

                    for o in range(NOCT):
                        wt = wpool.tile([128, WO], F16)
                        nc.sync.dma_start(out=wt[:],
                                          in_=w_d[:, o * WO:(o + 1) * WO])
                        psums = [ps.tile([128, 512], F32, name=f"pst{_i}", tag="pst")
                                 for _i in range(8)]
                        for idx in range(NMM):
                            slot, kyoff = _DESCS[idx]
                            for t in range(8):
                                b0 = slot + (t * 8 + kyoff) * 64
                                nc.tensor.matmul(
                                    psums[t][:],
                                    wt[:, idx * 128:(idx + 1) * 128],
                                    xt[:, b0:b0 + 512],
                                    start=(idx == 0), stop=(idx == NMM - 1))
                        for t in range(8):
                            if o == 0:
                                nc.scalar.activation(
                                    accs[t][:], psums[t][:],
                                    mybir.ActivationFunctionType.Square)
                            else:
                                sq = sqp.tile([128, 512], F32)
                                nc.scalar.activation(
                                    sq[:], psums[t][:],
                                    mybir.ActivationFunctionType.Square)
                                nc.vector.tensor_add(accs[t][:], accs[t][:],
                                                     sq[:])

                    for t in range(8):
                        tmp = foldp.tile([COUT, 512], F32)
                        nc.vector.tensor_copy(tmp[:], accs[t][64:128, :])
                        f = foldp.tile([COUT, 512], F32)
                        nc.vector.scalar_tensor_tensor(
                            f[:], tmp[:], bt[:], accs[t][0:64, :],
                            op0=mybir.AluOpType.add, op1=mybir.AluOpType.add)
                        nc.sync.dma_start(
                            out=out_d[n, :, t * 512:(t + 1) * 512], in_=f[:])

    nc.compile()
    _PROGRAM_CACHE[key] = nc
    return nc


_RUNNER_CACHE = {}


def _make_runner(nc):
    """Build a reusable jitted 8-core executor for the program `nc`.

    Mirrors bass2jax.run_bass_via_pjrt's multi-core path, but keeps the
    jitted shard_map alive so repeat calls don't re-trace/re-compile.
    """
    import jax
    from jax.experimental.shard_map import shard_map
    from jax.sharding import Mesh, PartitionSpec
    from concourse import bass2jax, mybir

    bass2jax.install_neuronx_cc_hook()

    partition_name = (nc.partition_id_tensor.name
                      if nc.partition_id_tensor else None)
    in_names, out_names, out_avals, out_shapes = [], [], [], []
    for alloc in nc.m.functions[0].allocations:
        if not isinstance(alloc, mybir.MemoryLocationSet):
            continue
        name = alloc.memorylocations[0].name
        if alloc.kind == "ExternalInput":
            if name != partition_name:
                in_names.append(name)
        elif alloc.kind == "ExternalOutput":
            shape = tuple(alloc.tensor_shape)
            dtype = mybir.dt.np(alloc.dtype)
            out_names.append(name)
            out_avals.append(jax.core.ShapedArray(shape, dtype))
            out_shapes.append((shape, dtype))
    n_params = len(in_names)
    n_outs = len(out_names)
    all_in_names = list(in_names) + list(out_names)
    if partition_name is not None:
        all_in_names.append(partition_name)
    donate = tuple(range(n_params, n_params + n_outs))

    def _body(*args):
        operands = list(args)
        if partition_name is not None:
            operands.append(bass2jax.partition_id_tensor())
        outs = bass2jax._bass_exec_p.bind(
            *operands,
            out_avals=tuple(out_avals),
            in_names=tuple(all_in_names),
            out_names=tuple(out_names),
            lowering_input_output_aliases=(),
            sim_require_finite=True,
            sim_require_nnan=True,
            nc=nc,
        )
        return tuple(outs)

    devices = jax.devices()[:N_CORES]
    mesh = Mesh(np.asarray(devices), ("core",))
    in_specs = (PartitionSpec("core"),) * (n_params + n_outs)
    out_specs = (PartitionSpec("core"),) * n_outs
    sharded = jax.jit(
        shard_map(_body, mesh=mesh, in_specs=in_specs, out_specs=out_specs,
                  check_rep=False),
        donate_argnums=donate, keep_unused=True)

    from jax.sharding import NamedSharding
    core_sharding = NamedSharding(mesh, PartitionSpec("core"))
    dev_cache = {}

    def run(in_maps, cache_key=None):
        if cache_key is not None and cache_key in dev_cache:
            concat_in = dev_cache[cache_key]
        else:
            concat_in = [
                jax.device_put(
                    np.concatenate([np.asarray(in_maps[c][name])
                                    for c in range(N_CORES)], axis=0),
                    core_sharding)
                for name in in_names]
            if cache_key is not None:
                dev_cache[cache_key] = concat_in
        concat_zeros = [
            np.zeros((N_CORES * s[0],) + tuple(s[1:]), d)
            for (s, d) in out_shapes]
        out_arrs = sharded(*concat_in, *concat_zeros)
        return [
            {name: np.asarray(out_arrs[i]).reshape(
                (N_CORES,) + out_shapes[i][0])[c]
             for i, name in enumerate(out_names)}
            for c in range(N_CORES)]

    return run


def _run(nc, xflat, w_flat, b_col, cache_key=None):
    runner = _RUNNER_CACHE.get(id(nc))
    if runner is None:
        runner = _make_runner(nc)
        _RUNNER_CACHE[id(nc)] = runner
    in_maps = []
    for c in range(N_CORES):
        in_maps.append({
            "x": np.ascontiguousarray(xflat[c * NIMG:(c + 1) * NIMG]),
            "w": w_flat,
            "b": b_col,
        })
    results = runner(in_maps, cache_key=cache_key)
    out = np.concatenate(
        [results[c]["out"].reshape(NIMG, COUT, H, W)
         for c in range(N_CORES)], axis=0)
    return out


def kernel(x, T_real, T_imag, w_r, w_i, b, _repeat=1):
    x = np.asarray(x, np.float32)
    xflat, w_flat, b_col = _host_prep(
        x, np.asarray(T_real, np.float32), np.asarray(T_imag, np.float32),
        np.asarray(w_r, np.float32), np.asarray(w_i, np.float32), b)
    nc = _build_program(repeat=_repeat)
    return _run(nc, xflat, w_flat, b_col)
